# revision 40
# baseline (speedup 1.0000x reference)
import numpy as np
import ml_dtypes

N = 50000
F = 64
E = 128
Q = 8
S = 2048
NC = 8
NPC = N // NC          # 6250 clauses per core
NPAD = 6400            # 25 * 256
NSC = 25               # super-chunks of 256 clauses (DoubleRow)
NCHUNK = 50            # 128-chunks
SB = 4                 # psum banks of 512 steps each
ST = 32                # stationary cols: Ehi(8) Elo(8) Ghi(8) Glo(8)
GS = 0.125             # scale on x for G so fp8 never saturates (|G|<448)
ENTROPY_COEF = 0.1
NG = 5                 # super-chunk groups for PE interleaving
GSC = NSC // NG        # super-chunks per group

_PROG = None


def _build_prog():
    import sys
    if "/opt/trn_rl_repo" not in sys.path:
        sys.path.insert(0, "/opt/trn_rl_repo")
    from concourse import bass, bacc, tile, mybir

    f32 = mybir.dt.float32
    bf16 = mybir.dt.bfloat16
    f8 = mybir.dt.float8e4
    AF = mybir.ActivationFunctionType
    ALU = mybir.AluOpType
    DR = mybir.MatmulPerfMode.DoubleRow

    nc = bacc.Bacc("TRN2", enable_partition_id=False)
    w1_d = nc.dram_tensor("w1", [F + 1, E], bf16, kind="ExternalInput")
    fv8_d = nc.dram_tensor("fv8", [F + 1, NPAD], f8, kind="ExternalInput")
    wb_d = nc.dram_tensor("wb", [E, Q + 1], f32, kind="ExternalInput")
    maskT_d = nc.dram_tensor("maskT", [128, NSC, 2, S], f8, kind="ExternalInput")
    stats_d = nc.dram_tensor("stats", [ST, S], f32, kind="ExternalOutput")
    xall_d = nc.dram_tensor("xall", [E, NCHUNK * Q], f32, kind="ExternalOutput")

    with tile.TileContext(nc) as tc:
        with (
            tc.tile_pool(name="const", bufs=1) as constp,
            tc.tile_pool(name="big", bufs=1) as bigp,
            tc.tile_pool(name="mask", bufs=1) as maskp,
            tc.tile_pool(name="ps", bufs=1, space=bass.MemorySpace.PSUM) as ps,
        ):
            wb_sb = constp.tile([E, Q + 1], f32)
            k2t_sb = constp.tile([E, Q], bf16)
            scr_sb = constp.tile([1, 1], f32)
            warm_sb = constp.tile([128, 640], bf16)

            w1_sb = constp.tile([F + 1, E], bf16)
            fv8_sb = bigp.tile([F + 1, NPAD], f8)
            # hT lives in one tile per hT-block-group: tile deps are coarse
            # (a reader waits ALL previously-emitted writers of the tile), so
            # a single hT tile would serialize every x' group on the latest
            # relu. Per-group tiles keep x'(g) waiting only on its own blocks.
            HB = [(0, 3), (3, 2), (5, 3), (8, 2), (10, 3)]
            ht_tiles = [bigp.tile([E, 512 * nj], bf16, tag=f"ht{g}",
                                  name=f"ht{g}")
                        for g, (j0, nj) in enumerate(HB)]

            def ht_ap(c):
                # lhsT slice for 128-chunk c inside its group tile
                j = c // 4
                for g, (j0, nj) in enumerate(HB):
                    if j0 <= j < j0 + nj:
                        lc = 128 * c - 512 * j0
                        return ht_tiles[g][:, lc:lc + 128]
            xall_sb = bigp.tile([E, NCHUNK * Q], f32)
            e_sb = bigp.tile([E, NSC, 2, Q], f32)
            xs_sb = bigp.tile([E, NSC, 2, Q], f32)
            g_sb = bigp.tile([E, NSC, 2, Q], f32)
            ehi_sb = bigp.tile([E, NSC, 2, Q], f32)
            ghi_sb = bigp.tile([E, NSC, 2, Q], f32)
            stat_sb = bigp.tile([E, NSC, 2, ST], f8)
            stats_sb = bigp.tile([ST, SB * 512], f32)

            # Everything rides the single SP ring, in priority order: other
            # arrangements measured worse — a second HWDGE ring gets starved
            # unpredictably (wb once landed at 24us), and SWDGE fv8 trickles
            # at ~56GB/s while stealing engine slots from the mask stream.
            # FIFO on one ring is deterministic: fv8/w1/wb first (0.43MB, the
            # embedder needs them at ~10us), then the mask stream.
            nc.sync.dma_start(fv8_sb[:, 0:1920], fv8_d[:, 0:1920])
            nc.sync.dma_start(w1_sb[:], w1_d[:])
            nc.sync.dma_start(wb_sb[:], wb_d[:])
            nc.sync.dma_start(fv8_sb[:, 1920:NPAD], fv8_d[:, 1920:NPAD])

            # Mask stream: big blocks early (they arrive while PE still does
            # the embedder), single-super-chunk blocks at the tail so the
            # last stats matmuls wait on only 512KB. Super-chunk 24's second
            # half is pure padding (clauses 6272-6399) and is never shipped:
            # its stats matmul runs as plain fp8 with K=128 instead.
            MBLK = [(0, 5), (5, 5), (10, 4), (14, 3), (17, 2), (19, 2),
                    (21, 1), (22, 1), (23, 1)]
            mts = {}
            for s0, w in MBLK:
                mt = maskp.tile([128, w, 2, S], f8, tag=f"m{s0}")
                nc.sync.dma_start(mt[:], maskT_d[:, s0:s0 + w, :, :])
                mts[s0] = (w, mt)
            # sc24 first half only (second half is pure padding). MUST stay
            # 128 partitions: a [106,...] DMA hit a degenerate descriptor
            # split (all 106 descs on 2 engines, 5.8us transfer); 128-desc
            # instructions round-robin evenly over all 16 engines.
            m24 = maskp.tile([128, 1, S], f8, tag="m24")
            nc.sync.dma_start(m24[:], maskT_d[:, 24, 0:1, :])

            def mslice(sc, b):
                for s0, (w, mt) in mts.items():
                    if s0 <= sc < s0 + w:
                        return mt[:, sc - s0, :, 512 * b:512 * (b + 1)]

            # ACT absorber for the wb DMA semaphore, then k2t copy on ACT so
            # the px matmuls depend on a single engine (ACT) only. The dummy
            # Exp prefetches the ACT exp table (1.3us load) off the critical
            # path — otherwise the first prep-group exp stalls on it.
            nc.scalar.activation(scr_sb[:], wb_sb[0:1, 0:1], AF.Relu)
            nc.scalar.activation(scr_sb[:], wb_sb[0:1, 0:1], AF.Exp)
            nc.scalar.activation(k2t_sb[:], wb_sb[:, 0:Q], AF.Copy)

            # PE p-state warm-up: the tensor engine ramps 1.2->2.4 GHz only
            # after ~3us of continuous execution. Run dummy matmuls on a
            # zeroed scratch while waiting for fw so the real work starts hot.
            nc.vector.memset(warm_sb[:], 0.0)
            xps = ps.tile([E, 512], f32, tag="x", bufs=1, name="x")
            # Light warm-up only: full-array warm-ups DO flip the HAM clock
            # gate to 2.4 GHz by ~11us (it watches array activity — 1x1
            # dummies never flip it), but all three runs with them measured
            # 55.7-58.3us vs 52.5 with light warm-up: the hot PE appears to
            # slow the DMA engines (shared power budget), and the kernel is
            # stream-bound, so a cool PE wins.
            for i in range(12):
                nc.tensor.matmul(xps[0:1, 0:256], warm_sb[0:1, 0:1],
                                 warm_sb[0:1, 0:256], start=True, stop=True)

            # hT = relu(W1x.T @ fv8x)  [E, NPAD] bf16 — b1 folded into the
            # GEMM as a 65th row (fv8x row 64 == 1.0), so the psum eviction
            # is a plain relu and can alternate ACT/DVE (halves its pacing).
            def emit_ht(j):
                c0 = 512 * j
                cw = min(512, NPAD - c0)
                g = next(g for g, (j0, nj) in enumerate(HB) if j0 <= j < j0 + nj)
                lc = c0 - 512 * HB[g][0]
                dst = ht_tiles[g][:, lc:lc + cw]
                ph = ps.tile([E, 512], f32, tag="w", bufs=3, name="w")
                nc.tensor.matmul(ph[:, :cw], w1_sb[:], fv8_sb[:, c0:c0 + cw],
                                 start=True, stop=True)
                if j % 2 == 0:
                    nc.scalar.activation(dst, ph[:, :cw], AF.Relu)
                else:
                    nc.vector.tensor_scalar_max(dst, ph[:, :cw], 0.0)

            stats_ps = [ps.tile([ST, 512], f32, tag=f"s{b}", bufs=1, name=f"s{b}")
                        for b in range(SB)]

            def emit_x(g):
                # x'[n,q] = hT_chunk.T @ K2T (c_q dropped: softmax shift-inv.)
                for c in range(2 * GSC * g, 2 * GSC * (g + 1)):
                    nc.tensor.matmul(xps[:, Q * c:Q * (c + 1)],
                                     ht_ap(c), k2t_sb[:],
                                     start=True, stop=True)

            def emit_prep(g):
                sl = slice(2 * GSC * Q * g, 2 * GSC * Q * (g + 1))
                sc = slice(GSC * g, GSC * (g + 1))
                nc.scalar.activation(e_sb[:, sc, :, :], xps[:, sl], AF.Exp)
                nc.scalar.activation(xs_sb[:, sc, :, :], xps[:, sl], AF.Copy,
                                     scale=GS)
                nc.vector.tensor_tensor(g_sb[:, sc], xs_sb[:, sc], e_sb[:, sc],
                                        ALU.mult)
                # hi/lo fp8 split: value = hi + lo, ~2^-8 combined rel err
                nc.vector.tensor_copy(stat_sb[:, sc, :, 0:Q], e_sb[:, sc])
                nc.vector.tensor_copy(ehi_sb[:, sc], stat_sb[:, sc, :, 0:Q])
                nc.vector.tensor_tensor(stat_sb[:, sc, :, Q:2 * Q], e_sb[:, sc],
                                        ehi_sb[:, sc], ALU.subtract)
                nc.vector.tensor_copy(stat_sb[:, sc, :, 2 * Q:3 * Q], g_sb[:, sc])
                nc.vector.tensor_copy(ghi_sb[:, sc], stat_sb[:, sc, :, 2 * Q:3 * Q])
                nc.vector.tensor_tensor(stat_sb[:, sc, :, 3 * Q:4 * Q], g_sb[:, sc],
                                        ghi_sb[:, sc], ALU.subtract)

            def emit_stats(g, final=False):
                # stats[32,S] += stat_chunk.T @ maskT_chunk, fp8 DoubleRow K=256
                for k in range(GSC * g, GSC * (g + 1)):
                    last = final and k == NSC - 1
                    for b in range(SB):
                        if k == NSC - 1:
                            # half super-chunk: second half is padding, K=128
                            nc.tensor.matmul(stats_ps[b][:, :],
                                             stat_sb[:, k, 0, :],
                                             m24[:, 0, 512 * b:512 * (b + 1)],
                                             start=False, stop=last,
                                             skip_group_check=True)
                        else:
                            nc.tensor.matmul(stats_ps[b][:, :],
                                             stat_sb[:, k, :, :],
                                             mslice(k, b), start=(k == 0),
                                             stop=last, perf_mode=DR,
                                             skip_group_check=True)
                        if last:
                            # per-bank evacuation split over ACT and DVE;
                            # separate psum tiles keep banks independent
                            dst = stats_sb[:, 512 * b:512 * (b + 1)]
                            if b % 2 == 0:
                                nc.scalar.activation(dst, stats_ps[b][:, :],
                                                     AF.Copy)
                            else:
                                nc.vector.tensor_copy(dst, stats_ps[b][:, :])

            # Interleave x'/prep groups into the hT loop, sliding each x'(g)
            # one hT-block-group later than its data needs: the relu evac of
            # a block lands ~1.4us after its matmul, so the extra blocks of
            # matmul work in between hide that latency and PE stays dense
            # (keeps the HAM clock warm). Stats matmuls stay at the end:
            # they'd stall the in-order PE stream waiting for mask blocks.
            def emit_ht_group(g):
                j0, nj = HB[g]
                for j in range(j0, j0 + nj):
                    emit_ht(j)
            emit_ht_group(0)
            emit_ht_group(1)
            emit_x(0); emit_prep(0)
            emit_ht_group(2)
            emit_x(1); emit_prep(1)
            emit_ht_group(3)
            emit_x(2); emit_prep(2)
            emit_ht_group(4)
            emit_x(3); emit_prep(3)
            emit_x(4); emit_prep(4)
            emit_stats(0)
            emit_stats(1)
            # xall is final now; ship it on the SP ring — FIFO puts its
            # descriptors behind the whole mask stream, so it transfers in
            # the tail shadow instead of stealing mid-stream bandwidth
            nc.scalar.activation(xall_sb[:], xps[:, 0:NCHUNK * Q], AF.Copy)
            nc.sync.dma_start(xall_d[:], xall_sb[:])
            emit_stats(2)
            emit_stats(3)
            emit_stats(4, final=True)
            # halves on different rings so the two dispatches overlap (the
            # mask stream is over, so cross-queue starvation can't bite)
            nc.sync.dma_start(stats_d[:, 0:1024], stats_sb[:, 0:1024])
            nc.scalar.dma_start(stats_d[:, 1024:2048], stats_sb[:, 1024:2048])

    nc.finalize()
    return nc


def _get_prog():
    global _PROG
    if _PROG is None:
        _PROG = _build_prog()
    return _PROG


def _prep(feature_vecs, W1, b1, W2, b2, keys, mask):
    m8 = mask.view(np.uint8) if mask.dtype == np.bool_ else mask.astype(np.uint8)
    m8 = m8 * np.uint8(0x38)               # fp8e4m3 bit pattern of 1.0
    mT = np.ascontiguousarray(m8.T)        # [N, S]

    wb = np.zeros((E, Q + 1), np.float32)
    wb[:, 0:Q] = (np.asarray(W2, np.float64) @ np.asarray(keys, np.float64).T
                  ).astype(np.float32)     # K2T[e,q]

    # b1 folded into the GEMM as row F of W1 (paired with a 1.0 row in fv8)
    w1b = np.concatenate([np.asarray(W1, np.float32),
                          np.asarray(b1, np.float32)[None, :]]
                         ).astype(ml_dtypes.bfloat16)

    in_maps = []
    for d in range(NC):
        sl = slice(d * NPC, (d + 1) * NPC)
        fv8 = np.zeros((F + 1, NPAD), ml_dtypes.float8_e4m3)
        fv8[0:F, 0:NPC] = feature_vecs[sl].T.astype(ml_dtypes.float8_e4m3)
        fv8[F, 0:NPC] = 1.0
        mt = np.zeros((NPAD, S), np.uint8)
        mt[:NPC] = mT[sl]
        mt4 = np.ascontiguousarray(
            mt.reshape(NSC, 2, 128, S).transpose(2, 0, 1, 3))
        in_maps.append({
            "w1": w1b,
            "fv8": fv8,
            "wb": wb,
            "maskT": mt4.view(ml_dtypes.float8_e4m3),
        })
    return in_maps


def kernel(feature_vecs, W1, b1, W2, b2, keys, rewards, mask, queue_idx, sel_idx):
    import sys
    if "/opt/trn_rl_repo" not in sys.path:
        sys.path.insert(0, "/opt/trn_rl_repo")
    from concourse.bass_utils import run_bass_kernel_spmd

    nc = _get_prog()
    in_maps = _prep(feature_vecs, W1, b1, W2, b2, keys, mask)
    res = run_bass_kernel_spmd(nc, in_maps, list(range(NC))).results

    qs = np.asarray(queue_idx).astype(np.int64)
    ar = np.arange(S)
    Z = np.zeros(S, np.float64)
    S1 = np.zeros(S, np.float64)
    cnt = np.asarray(mask).sum(axis=1, dtype=np.float64)
    for d in range(NC):
        st = res[d]["stats"].astype(np.float64)
        Z += st[qs, ar] + st[Q + qs, ar]
        S1 += st[2 * Q + qs, ar] + st[3 * Q + qs, ar]
    S1 /= GS

    xall = np.stack([res[d]["xall"] for d in range(NC)]).astype(np.float64)
    sel = np.asarray(sel_idx).astype(np.int64)
    d_arr = sel // NPC
    nloc = sel % NPC
    x_sel = xall[d_arr, nloc % 128, (nloc // 128) * Q + qs]

    logZ = np.log(Z)
    ce = logZ - x_sel
    me = (S1 / Z - logZ) / np.log(cnt)
    loss = (np.asarray(rewards, np.float64) * ce).sum() + ENTROPY_COEF * me.sum()
    return np.array([loss], dtype=np.float32)


# revision 44
# speedup vs baseline: 1.0688x; 1.0688x over previous
import numpy as np
import ml_dtypes

N = 50000
F = 64
E = 128
Q = 8
S = 2048
NC = 8
NPC = N // NC          # 6250 clauses per core
NPAD = 6400            # 25 * 256
NSC = 25               # super-chunks of 256 clauses (DoubleRow)
NCHUNK = 50            # 128-chunks
SB = 4                 # psum banks of 512 steps each
ST = 32                # stationary cols: Ehi(8) Elo(8) Ghi(8) Glo(8)
GS = 0.125             # scale on x for G so fp8 never saturates (|G|<448)
ENTROPY_COEF = 0.1
NG = 5                 # super-chunk groups for PE interleaving
GSC = NSC // NG        # super-chunks per group

_PROG = None


def _build_prog():
    import sys
    if "/opt/trn_rl_repo" not in sys.path:
        sys.path.insert(0, "/opt/trn_rl_repo")
    from concourse import bass, bacc, tile, mybir

    f32 = mybir.dt.float32
    bf16 = mybir.dt.bfloat16
    f8 = mybir.dt.float8e4
    AF = mybir.ActivationFunctionType
    ALU = mybir.AluOpType
    DR = mybir.MatmulPerfMode.DoubleRow

    nc = bacc.Bacc("TRN2", enable_partition_id=False)
    w1_d = nc.dram_tensor("w1", [F + 1, E], bf16, kind="ExternalInput")
    fv8_d = nc.dram_tensor("fv8", [F + 1, NPAD], f8, kind="ExternalInput")
    wb_d = nc.dram_tensor("wb", [E, Q + 1], f32, kind="ExternalInput")
    maskT_d = nc.dram_tensor("maskT", [128, NSC, 2, S], f8, kind="ExternalInput")
    stats_d = nc.dram_tensor("stats", [ST, S], f32, kind="ExternalOutput")
    xall_d = nc.dram_tensor("xall", [E, NCHUNK * Q], f32, kind="ExternalOutput")

    with tile.TileContext(nc) as tc:
        with (
            tc.tile_pool(name="const", bufs=1) as constp,
            tc.tile_pool(name="big", bufs=1) as bigp,
            tc.tile_pool(name="mask", bufs=1) as maskp,
            tc.tile_pool(name="ps", bufs=1, space=bass.MemorySpace.PSUM) as ps,
        ):
            wb_sb = constp.tile([E, Q + 1], f32)
            k2t_sb = constp.tile([E, Q], bf16)
            scr_sb = constp.tile([1, 1], f32)
            warm_sb = constp.tile([128, 640], bf16)

            w1_sb = constp.tile([F + 1, E], bf16)
            fv8_sb = bigp.tile([F + 1, NPAD], f8)
            # hT lives in one tile per hT-block-group: tile deps are coarse
            # (a reader waits ALL previously-emitted writers of the tile), so
            # a single hT tile would serialize every x' group on the latest
            # relu. Per-group tiles keep x'(g) waiting only on its own blocks.
            HB = [(0, 3), (3, 2), (5, 3), (8, 2), (10, 3)]
            ht_tiles = [bigp.tile([E, 512 * nj], bf16, tag=f"ht{g}",
                                  name=f"ht{g}")
                        for g, (j0, nj) in enumerate(HB)]

            def ht_ap(c):
                # lhsT slice for 128-chunk c inside its group tile
                j = c // 4
                for g, (j0, nj) in enumerate(HB):
                    if j0 <= j < j0 + nj:
                        lc = 128 * c - 512 * j0
                        return ht_tiles[g][:, lc:lc + 128]
            xall_sb = bigp.tile([E, NCHUNK * Q], f32)
            e_sb = bigp.tile([E, NSC, 2, Q], f32)
            xs_sb = bigp.tile([E, NSC, 2, Q], f32)
            g_sb = bigp.tile([E, NSC, 2, Q], f32)
            ehi_sb = bigp.tile([E, NSC, 2, Q], f32)
            ghi_sb = bigp.tile([E, NSC, 2, Q], f32)
            stat_sb = bigp.tile([E, NSC, 2, ST], f8)
            stats_sb = bigp.tile([ST, SB * 512], f32)

            # Everything rides the single SP ring, in priority order: other
            # arrangements measured worse — a second HWDGE ring gets starved
            # unpredictably (wb once landed at 24us), and SWDGE fv8 trickles
            # at ~56GB/s while stealing engine slots from the mask stream.
            # FIFO on one ring is deterministic: fv8/w1/wb first (0.43MB, the
            # embedder needs them at ~10us), then the mask stream.
            nc.sync.dma_start(fv8_sb[:, 0:2048], fv8_d[:, 0:2048])
            nc.sync.dma_start(w1_sb[:], w1_d[:])
            nc.sync.dma_start(wb_sb[:], wb_d[:])

            # Mask stream: big blocks early (they arrive while PE still does
            # the embedder), single-super-chunk blocks at the tail so the
            # last stats matmuls wait on only 512KB. Super-chunk 24's second
            # half is pure padding (clauses 6272-6399) and is never shipped:
            # its stats matmul runs as plain fp8 with K=128 instead.
            # block0 rides ahead of the rest of fv8: stats(0) is emitted
            # before hT4, so the PE chews block0 while fv8b still streams —
            # the mask's first byte moves ~2.5us earlier.
            MBLK = [(0, 5), (5, 5), (10, 4), (14, 3), (17, 2), (19, 2),
                    (21, 1), (22, 1), (23, 1)]
            mts = {}
            for s0, w in MBLK:
                mt = maskp.tile([128, w, 2, S], f8, tag=f"m{s0}")
                nc.sync.dma_start(mt[:], maskT_d[:, s0:s0 + w, :, :])
                mts[s0] = (w, mt)
                if s0 == 0:
                    nc.sync.dma_start(fv8_sb[:, 2048:NPAD],
                                      fv8_d[:, 2048:NPAD])
            # sc24 first half only (second half is pure padding). MUST stay
            # 128 partitions: a [106,...] DMA hit a degenerate descriptor
            # split (all 106 descs on 2 engines, 5.8us transfer); 128-desc
            # instructions round-robin evenly over all 16 engines.
            m24 = maskp.tile([128, 1, S], f8, tag="m24")
            nc.sync.dma_start(m24[:], maskT_d[:, 24, 0:1, :])

            def mslice(sc, b):
                for s0, (w, mt) in mts.items():
                    if s0 <= sc < s0 + w:
                        return mt[:, sc - s0, :, 512 * b:512 * (b + 1)]

            # ACT absorber for the wb DMA semaphore, then k2t copy on ACT so
            # the px matmuls depend on a single engine (ACT) only. The dummy
            # Exp prefetches the ACT exp table (1.3us load) off the critical
            # path — otherwise the first prep-group exp stalls on it.
            nc.scalar.activation(scr_sb[:], wb_sb[0:1, 0:1], AF.Relu)
            nc.scalar.activation(scr_sb[:], wb_sb[0:1, 0:1], AF.Exp)
            nc.scalar.activation(k2t_sb[:], wb_sb[:, 0:Q], AF.Copy)

            # PE p-state warm-up: the tensor engine ramps 1.2->2.4 GHz only
            # after ~3us of continuous execution. Run dummy matmuls on a
            # zeroed scratch while waiting for fw so the real work starts hot.
            nc.vector.memset(warm_sb[:], 0.0)
            xps = ps.tile([E, 512], f32, tag="x", bufs=1, name="x")
            # FULL-ARRAY warm-up, ~3.8us continuous: the HAM clock gate
            # watches PE *array activity* (1x1 dummies never flipped it in
            # any trace — the flip always waited for the dense stats MMs at
            # ~27us). K=128/M=128/N=512 matmuls light the whole array, so
            # the clock un-throttles to 2.4 GHz during the warm-up itself
            # and the stats matmuls run at 216ns instead of 427ns cold.
            for i in range(9):
                nc.tensor.matmul(xps[:, :], warm_sb[:, 0:128],
                                 warm_sb[:, 128:640], start=True, stop=True)

            # hT = relu(W1x.T @ fv8x)  [E, NPAD] bf16 — b1 folded into the
            # GEMM as a 65th row (fv8x row 64 == 1.0), so the psum eviction
            # is a plain relu and can alternate ACT/DVE (halves its pacing).
            def emit_ht(j):
                c0 = 512 * j
                cw = min(512, NPAD - c0)
                g = next(g for g, (j0, nj) in enumerate(HB) if j0 <= j < j0 + nj)
                lc = c0 - 512 * HB[g][0]
                dst = ht_tiles[g][:, lc:lc + cw]
                ph = ps.tile([E, 512], f32, tag="w", bufs=3, name="w")
                nc.tensor.matmul(ph[:, :cw], w1_sb[:], fv8_sb[:, c0:c0 + cw],
                                 start=True, stop=True)
                if j % 2 == 0:
                    nc.scalar.activation(dst, ph[:, :cw], AF.Relu)
                else:
                    nc.vector.tensor_scalar_max(dst, ph[:, :cw], 0.0)

            stats_ps = [ps.tile([ST, 512], f32, tag=f"s{b}", bufs=1, name=f"s{b}")
                        for b in range(SB)]

            def emit_x(g):
                # x'[n,q] = hT_chunk.T @ K2T (c_q dropped: softmax shift-inv.)
                for c in range(2 * GSC * g, 2 * GSC * (g + 1)):
                    nc.tensor.matmul(xps[:, Q * c:Q * (c + 1)],
                                     ht_ap(c), k2t_sb[:],
                                     start=True, stop=True)

            def emit_prep(g):
                sl = slice(2 * GSC * Q * g, 2 * GSC * Q * (g + 1))
                sc = slice(GSC * g, GSC * (g + 1))
                nc.scalar.activation(e_sb[:, sc, :, :], xps[:, sl], AF.Exp)
                nc.scalar.activation(xs_sb[:, sc, :, :], xps[:, sl], AF.Copy,
                                     scale=GS)
                nc.vector.tensor_tensor(g_sb[:, sc], xs_sb[:, sc], e_sb[:, sc],
                                        ALU.mult)
                # hi/lo fp8 split: value = hi + lo, ~2^-8 combined rel err
                nc.vector.tensor_copy(stat_sb[:, sc, :, 0:Q], e_sb[:, sc])
                nc.vector.tensor_copy(ehi_sb[:, sc], stat_sb[:, sc, :, 0:Q])
                nc.vector.tensor_tensor(stat_sb[:, sc, :, Q:2 * Q], e_sb[:, sc],
                                        ehi_sb[:, sc], ALU.subtract)
                nc.vector.tensor_copy(stat_sb[:, sc, :, 2 * Q:3 * Q], g_sb[:, sc])
                nc.vector.tensor_copy(ghi_sb[:, sc], stat_sb[:, sc, :, 2 * Q:3 * Q])
                nc.vector.tensor_tensor(stat_sb[:, sc, :, 3 * Q:4 * Q], g_sb[:, sc],
                                        ghi_sb[:, sc], ALU.subtract)

            def emit_stats(g, final=False):
                # stats[32,S] += stat_chunk.T @ maskT_chunk, fp8 DoubleRow K=256
                for k in range(GSC * g, GSC * (g + 1)):
                    last = final and k == NSC - 1
                    for b in range(SB):
                        if k == NSC - 1:
                            # half super-chunk: second half is padding, K=128
                            nc.tensor.matmul(stats_ps[b][:, :],
                                             stat_sb[:, k, 0, :],
                                             m24[:, 0, 512 * b:512 * (b + 1)],
                                             start=False, stop=last,
                                             skip_group_check=True)
                        else:
                            nc.tensor.matmul(stats_ps[b][:, :],
                                             stat_sb[:, k, :, :],
                                             mslice(k, b), start=(k == 0),
                                             stop=last, perf_mode=DR,
                                             skip_group_check=True)
                        if last:
                            # per-bank evacuation split over ACT and DVE;
                            # separate psum tiles keep banks independent
                            dst = stats_sb[:, 512 * b:512 * (b + 1)]
                            if b % 2 == 0:
                                nc.scalar.activation(dst, stats_ps[b][:, :],
                                                     AF.Copy)
                            else:
                                nc.vector.tensor_copy(dst, stats_ps[b][:, :])

            # Interleave x'/prep groups into the hT loop, sliding each x'(g)
            # one hT-block-group later than its data needs: the relu evac of
            # a block lands ~1.4us after its matmul, so the extra blocks of
            # matmul work in between hide that latency and PE stays dense
            # (keeps the HAM clock warm). Stats matmuls stay at the end:
            # they'd stall the in-order PE stream waiting for mask blocks.
            # Group 0's whole chain (x'/prep/stats) runs off fv8's first
            # 2048 cols (hT blocks 0-3) BEFORE hT4, which waits for the rest
            # of fv8 behind mask block0 on the ring. Per-group ht tiles keep
            # the later x' groups waiting only on their own blocks' relus.
            for j in range(4):
                emit_ht(j)
            emit_x(0); emit_prep(0)
            emit_stats(0)
            for j in range(4, 13):
                emit_ht(j)
            emit_x(1); emit_prep(1)
            emit_x(2); emit_prep(2)
            emit_x(3); emit_prep(3)
            emit_x(4); emit_prep(4)
            emit_stats(1)
            # xall is final now; ship it on the SP ring — FIFO puts its
            # descriptors behind the whole mask stream, so it transfers in
            # the tail shadow instead of stealing mid-stream bandwidth
            nc.scalar.activation(xall_sb[:], xps[:, 0:NCHUNK * Q], AF.Copy)
            nc.sync.dma_start(xall_d[:], xall_sb[:])
            emit_stats(2)
            emit_stats(3)
            emit_stats(4, final=True)
            # halves on different rings so the two dispatches overlap (the
            # mask stream is over, so cross-queue starvation can't bite)
            nc.sync.dma_start(stats_d[:, 0:1024], stats_sb[:, 0:1024])
            nc.scalar.dma_start(stats_d[:, 1024:2048], stats_sb[:, 1024:2048])

    nc.finalize()
    return nc


def _get_prog():
    global _PROG
    if _PROG is None:
        _PROG = _build_prog()
    return _PROG


def _prep(feature_vecs, W1, b1, W2, b2, keys, mask):
    m8 = mask.view(np.uint8) if mask.dtype == np.bool_ else mask.astype(np.uint8)
    m8 = m8 * np.uint8(0x38)               # fp8e4m3 bit pattern of 1.0
    mT = np.ascontiguousarray(m8.T)        # [N, S]

    wb = np.zeros((E, Q + 1), np.float32)
    wb[:, 0:Q] = (np.asarray(W2, np.float64) @ np.asarray(keys, np.float64).T
                  ).astype(np.float32)     # K2T[e,q]

    # b1 folded into the GEMM as row F of W1 (paired with a 1.0 row in fv8)
    w1b = np.concatenate([np.asarray(W1, np.float32),
                          np.asarray(b1, np.float32)[None, :]]
                         ).astype(ml_dtypes.bfloat16)

    in_maps = []
    for d in range(NC):
        sl = slice(d * NPC, (d + 1) * NPC)
        fv8 = np.zeros((F + 1, NPAD), ml_dtypes.float8_e4m3)
        fv8[0:F, 0:NPC] = feature_vecs[sl].T.astype(ml_dtypes.float8_e4m3)
        fv8[F, 0:NPC] = 1.0
        mt = np.zeros((NPAD, S), np.uint8)
        mt[:NPC] = mT[sl]
        mt4 = np.ascontiguousarray(
            mt.reshape(NSC, 2, 128, S).transpose(2, 0, 1, 3))
        in_maps.append({
            "w1": w1b,
            "fv8": fv8,
            "wb": wb,
            "maskT": mt4.view(ml_dtypes.float8_e4m3),
        })
    return in_maps


def kernel(feature_vecs, W1, b1, W2, b2, keys, rewards, mask, queue_idx, sel_idx):
    import sys
    if "/opt/trn_rl_repo" not in sys.path:
        sys.path.insert(0, "/opt/trn_rl_repo")
    from concourse.bass_utils import run_bass_kernel_spmd

    nc = _get_prog()
    in_maps = _prep(feature_vecs, W1, b1, W2, b2, keys, mask)
    res = run_bass_kernel_spmd(nc, in_maps, list(range(NC))).results

    qs = np.asarray(queue_idx).astype(np.int64)
    ar = np.arange(S)
    Z = np.zeros(S, np.float64)
    S1 = np.zeros(S, np.float64)
    cnt = np.asarray(mask).sum(axis=1, dtype=np.float64)
    for d in range(NC):
        st = res[d]["stats"].astype(np.float64)
        Z += st[qs, ar] + st[Q + qs, ar]
        S1 += st[2 * Q + qs, ar] + st[3 * Q + qs, ar]
    S1 /= GS

    xall = np.stack([res[d]["xall"] for d in range(NC)]).astype(np.float64)
    sel = np.asarray(sel_idx).astype(np.int64)
    d_arr = sel // NPC
    nloc = sel % NPC
    x_sel = xall[d_arr, nloc % 128, (nloc // 128) * Q + qs]

    logZ = np.log(Z)
    ce = logZ - x_sel
    me = (S1 / Z - logZ) / np.log(cnt)
    loss = (np.asarray(rewards, np.float64) * ce).sum() + ENTROPY_COEF * me.sum()
    return np.array([loss], dtype=np.float32)
